# revision 26
# baseline (speedup 1.0000x reference)
"""Trainium2 Bass kernel for nn_MinimalNetwork (equivariant GNN message passing).

v2: fp16 pipeline tuned for DVE 2x perf mode.

Sharded over 8 NeuronCores by edge (data parallel). Per core, per
512-edge super-tile (4 sub-tiles of 128 edges = partition dim):
  radial basis -> 3-layer silu MLP (fp16 matmuls, PE)
  R   = h^T @ W3perm   [128, 1216] fp16 (PE + copy)
  CY  = rsh @ CC2perm  [128, 259]  fp16 (PE + copy)
  D-stage: per-edge F x CY products (DVE 2x) + ii-reduction (DVE)
  R-stage: per-edge R x D products (DVE 2x) + m-reduction halving tree
  combine: sel @ msg matmul (host-built per-tile dedup matrices, PE)
  scatter: one 4-column indirect DMA per super into fp16 node table.
Host sums the 8 per-core tables in fp32.

Self-contained: shapes/layout hardcoded for the 200000-edge/12500-node
instance; works for any edge count that bin-packs into 128-edge tiles.
"""

import math
from contextlib import ExitStack
from itertools import accumulate

import numpy as np

# ----------------- problem constants (hardcoded) -----------------
N_NODES = 12500
N_EDGES = 200000
N_CORES = 8
LO = [0, 1, 2]            # l value per block (multiplicity 8 each)
SH_DIM = 25
FEAT_OFF = [0, 8, 32, 72]
FEAT_DIM = 72
N_BASIS, H = 10, 100
R_DIM = 1216
MIN_R, MAX_R = 0.7, 3.2
SWISH_SCALE = 1.679177
SUB = 128
SUPER = 512
N_SUB = SUPER // SUB


def _nl(i, j):
    return 2 * min(i, j) + 1


def _no(i):
    return 2 * i + 1


def _nj(j):
    return 2 * j + 1


M_I = [sum(8 * _nl(i, j) for j in range(3)) for i in range(3)]      # 24,56,72
IOFF = [0] + list(accumulate(8 * m for m in M_I))                    # R col blocks


def _moff(i, j):
    return sum(8 * _nl(i, j2) for j2 in range(j))


W_J = [sum(_no(i) * _nl(i, j) for i in range(3)) for j in range(3)]  # 9,25,35


def _woff(j, i):
    return sum(_no(i2) * _nl(i2, j) for i2 in range(i))


JOFF2 = [0] + list(accumulate(W_J[j] * _nj(j) for j in range(3)))    # cc2 col blocks
CY_DIM = JOFF2[-1]                                                   # 259

# original (reference) R block offsets: pairs (i,j) blocks of [u][v][k]
R_OFF = [0] + list(
    accumulate(8 * 8 * _nl(i, j) for i in range(3) for j in range(3))
)


def _cc_layout():
    layout, off = {}, 0
    for lo in LO:
        for li in LO:
            for lf in range(abs(lo - li), lo + li + 1):
                if (lo, li, lf) not in layout:
                    shp = (2 * lo + 1, 2 * li + 1, 2 * lf + 1)
                    layout[(lo, li, lf)] = (off, shp)
                    off += shp[0] * shp[1] * shp[2]
    return layout, off


CC_LAYOUT, CC_TOTAL = _cc_layout()  # 1225


def _norm_coef():
    nc = np.zeros((3, 3), dtype=np.float64)
    for i, lo in enumerate(LO):
        ns = sum(8 * (2 * min(lo, li) + 1) for li in LO)
        nc[i, :] = math.sqrt(4 * math.pi) * math.sqrt(2 * lo + 1) / math.sqrt(ns)
    return nc


NORM = _norm_coef()


def build_cc2(cc: np.ndarray) -> np.ndarray:
    """CC2 [25, CY_DIM]; column = JOFF2[j] + w*nj + ii, w = woff(j,i)+o*nl+k."""
    cc2 = np.zeros((SH_DIM, CY_DIM), dtype=np.float32)
    for j in range(3):
        nj = _nj(j)
        for i in range(3):
            nl, no = _nl(i, j), _no(i)
            for k, lf in enumerate(range(abs(i - j), i + j + 1)):
                if k >= nl:
                    break
                off, shp = CC_LAYOUT[(i, j, lf)]
                C = cc[off: off + shp[0] * shp[1] * shp[2]].reshape(shp)
                for o in range(no):
                    w = _woff(j, i) + o * nl + k
                    for ii in range(nj):
                        col = JOFF2[j] + w * nj + ii
                        cc2[lf * lf: lf * lf + 2 * lf + 1, col] = (
                            np.float32(NORM[i, j]) * C[o, ii, :]
                        )
    return cc2


def permute_w3(W3s: np.ndarray) -> np.ndarray:
    """Reorder W3 columns: newcol(i,u,j,v,k) = IOFF[i]+u*M_I[i]+moff(i,j)+v*nl+k."""
    perm = np.zeros(R_DIM, dtype=np.int64)
    for i in range(3):
        for j in range(3):
            nl = _nl(i, j)
            p = i * 3 + j
            for u in range(8):
                for v in range(8):
                    for k in range(nl):
                        old = R_OFF[p] + u * (8 * nl) + v * nl + k
                        new = IOFF[i] + u * M_I[i] + _moff(i, j) + v * nl + k
                        perm[new] = old
    return np.ascontiguousarray(W3s[:, perm])


def fold_weights(W0, W1, W2, W3):
    s = SWISH_SCALE
    return (
        (W0 / math.sqrt(N_BASIS)).astype(np.float16),
        (s * W1 / math.sqrt(H)).astype(np.float16),
        (s * W2 / math.sqrt(H)).astype(np.float16),
        permute_w3((s * W3 / math.sqrt(H)).astype(np.float32)).astype(np.float16),
    )


# ----------------- bass program -----------------

def build_program(n_super: int, n_nodes: int, debug: bool = False):
    import concourse.bass as bass
    import concourse.tile as tile
    from concourse import bacc, mybir

    f32 = mybir.dt.float32
    f16 = mybir.dt.float16
    i32 = mybir.dt.int32
    AF = mybir.ActivationFunctionType
    OP = mybir.AluOpType

    nc = bacc.Bacc()

    e_pad = n_super * SUPER
    n_out = n_nodes + 1

    rshT_d = nc.dram_tensor("rshT", [SH_DIM, e_pad], f16, kind="ExternalInput")
    radii_d = nc.dram_tensor("radii", [1, e_pad], f32, kind="ExternalInput")
    src_d = nc.dram_tensor("srcidx", [n_super, SUB, N_SUB], i32, kind="ExternalInput")
    dst_d = nc.dram_tensor("dstidx", [n_super, SUB, N_SUB], i32, kind="ExternalInput")
    sel_d = nc.dram_tensor("sel", [n_super, SUB, N_SUB * SUB], f16, kind="ExternalInput")
    feat_d = nc.dram_tensor("features", [n_out, FEAT_DIM], f16, kind="ExternalInput")
    w0_d = nc.dram_tensor("W0p", [N_BASIS, H], f16, kind="ExternalInput")
    w1_d = nc.dram_tensor("W1p", [H, H], f16, kind="ExternalInput")
    w2_d = nc.dram_tensor("W2p", [H, H], f16, kind="ExternalInput")
    w3_d = nc.dram_tensor("W3p", [H, R_DIM], f16, kind="ExternalInput")
    cc2_d = nc.dram_tensor("CC2", [SH_DIM, CY_DIM], f16, kind="ExternalInput")
    csc_d = nc.dram_tensor("cscale", [N_BASIS, 1], f32, kind="ExternalInput")
    cbi_d = nc.dram_tensor("cbias", [N_BASIS, 1], f32, kind="ExternalInput")
    out_d = nc.dram_tensor("out", [n_out, FEAT_DIM], f16, kind="ExternalOutput")
    if debug:
        dbg = {
            "dbg_fg": [SUB, N_SUB * FEAT_DIM],
            "dbg_r16": [SUB, N_SUB * R_DIM],
            "dbg_cy16": [SUB, N_SUB * CY_DIM],
            "dbg_d2_0": [SUB, N_SUB * 1 * 24],
            "dbg_d2_1": [SUB, N_SUB * 3 * 56],
            "dbg_d2_2": [SUB, N_SUB * 5 * 72],
            "dbg_rt_2": [SUB, N_SUB * 8 * 5 * 72],
            "dbg_msg": [SUB, N_SUB * FEAT_DIM],
            "dbg_msg2": [SUB, N_SUB * FEAT_DIM],
            "dbg_h": [H, SUPER],
        }
        dbg_d = {k: nc.dram_tensor(k, v, f16, kind="ExternalOutput")
                 for k, v in dbg.items()}

    with tile.TileContext(nc) as tc, ExitStack() as ctx:
        cpool = ctx.enter_context(tc.tile_pool(name="consts", bufs=1))
        inpool = ctx.enter_context(tc.tile_pool(name="in", bufs=3))
        hpool = ctx.enter_context(tc.tile_pool(name="h", bufs=2))
        dpool = ctx.enter_context(tc.tile_pool(name="xfer", bufs=2))
        wpool = ctx.enter_context(tc.tile_pool(name="work", bufs=1))
        d2pool = ctx.enter_context(tc.tile_pool(name="d2p", bufs=2))
        mpool = ctx.enter_context(tc.tile_pool(name="msg", bufs=2))
        ps_mlp = ctx.enter_context(tc.tile_pool(name="psmlp", bufs=1, space="PSUM"))
        ps_r = ctx.enter_context(tc.tile_pool(name="psr", bufs=1, space="PSUM"))
        ps_cy = ctx.enter_context(tc.tile_pool(name="pscy", bufs=1, space="PSUM"))
        ps_dred = ctx.enter_context(tc.tile_pool(name="psdred", bufs=1, space="PSUM"))

        w0_s = cpool.tile([N_BASIS, H], f16)
        w1_s = cpool.tile([H, H], f16)
        w2_s = cpool.tile([H, H], f16)
        w3_s = cpool.tile([H, R_DIM], f16)
        cc2_s = cpool.tile([SH_DIM, CY_DIM], f16)
        csc_s = cpool.tile([N_BASIS, 1], f32)
        cbi_s = cpool.tile([N_BASIS, 1], f32)
        ones_s = cpool.tile([1, N_BASIS], f32)
        ident_s = cpool.tile([SUB, SUB], f16)
        from concourse.masks import make_identity
        make_identity(nc, ident_s[:])
        for t, d in (
            (w0_s, w0_d), (w1_s, w1_d), (w2_s, w2_d), (w3_s, w3_d),
            (cc2_s, cc2_d), (csc_s, csc_d), (cbi_s, cbi_d),
        ):
            nc.sync.dma_start(t[:], d[:])
        nc.vector.memset(ones_s[:], 1.0)

        # zero-init output table
        nfull = n_out // SUB
        nc.sync.dma_start(
            out_d[: nfull * SUB, :].rearrange("(a p) c -> p a c", p=SUB),
            zero_s[:].unsqueeze(1).broadcast_to((SUB, nfull, FEAT_DIM)),
        )
        rem = n_out - nfull * SUB
        if rem:
            nc.sync.dma_start(out_d[nfull * SUB:, :], zero_s[:rem, :])

        C = N_SUB  # 4 sub-tiles folded per instruction

        lp = ctx.enter_context(
            nc.allow_low_precision(reason="fp16 pipeline within 2e-2 tolerance"))
        for s in range(n_super):
            e0 = s * SUPER
            # ---- loads ----
            rsh_t = inpool.tile([SH_DIM, SUPER], f16, tag="rsh")
            nc.sync.dma_start(rsh_t[:], rshT_d[:, e0:e0 + SUPER])
            rad_t = inpool.tile([1, SUPER], f32, tag="rad")
            nc.sync.dma_start(rad_t[:], radii_d[:, e0:e0 + SUPER])
            src_t = inpool.tile([SUB, C], i32, tag="src")
            nc.sync.dma_start(src_t[:], src_d[s])
            dst_t = inpool.tile([SUB, C], i32, tag="dst")
            nc.sync.dma_start(dst_t[:], dst_d[s])
            sel_t = inpool.tile([SUB, C * SUB], f16, tag="sel")
            nc.sync.dma_start(sel_t[:], sel_d[s])
            fg_t = inpool.tile([SUB, C * FEAT_DIM], f16, tag="fg")
            for c in range(C):
                nc.gpsimd.indirect_dma_start(
                    out=fg_t[:, c * FEAT_DIM:(c + 1) * FEAT_DIM],
                    out_offset=None,
                    in_=feat_d[:],
                    in_offset=bass.IndirectOffsetOnAxis(
                        ap=src_t[:, c:c + 1], axis=0),
                )

            # ---- radial basis (fp32 -> fp16), reusing the MLP PSUM bank ----
            rb_full = ps_mlp.tile([H, SUPER], f32, tag="hp", space="PSUM",
                                  name="rb")
            rb_ps = rb_full[0:N_BASIS, :]
            nc.tensor.matmul(rb_ps, ones_s[:], rad_t[:], start=True, stop=True)
            z2_t = hpool.tile([N_BASIS, SUPER], f32, tag="z2")
            nc.scalar.activation(
                z2_t[:], rb_ps, AF.Square, bias=cbi_s[:], scale=csc_s[:]
            )
            bas_t = hpool.tile([N_BASIS, SUPER], f16, tag="bas")
            nc.scalar.activation(bas_t[:], z2_t[:], AF.Exp, scale=-1.0)

            # ---- MLP (fp16 matmuls, silu out fp16) ----
            h = bas_t
            for li, w_s in enumerate((w0_s, w1_s, w2_s)):
                hp = ps_mlp.tile([H, SUPER], f32, tag="hp", space="PSUM")
                nc.tensor.matmul(hp[:], w_s[:], h[:], start=True, stop=True)
                hn = hpool.tile([H, SUPER], f16, tag=f"h{li}")
                nc.scalar.activation(hn[:], hp[:], AF.Silu)
                h = hn

            # ---- R and CY per sub-tile ----
            r16 = dpool.tile([SUB, C * R_DIM], f16, tag="r16")
            cy16 = dpool.tile([SUB, C * CY_DIM], f16, tag="cy16")
            for c in range(C):
                esl = slice(c * SUB, (c + 1) * SUB)
                r_ps = ps_r.tile([SUB, R_DIM], f32, tag="r", space="PSUM")
                for c0 in range(0, R_DIM, 512):
                    c1 = min(c0 + 512, R_DIM)
                    nc.tensor.matmul(
                        r_ps[:, c0:c1], h[:, esl], w3_s[:, c0:c1],
                        start=True, stop=True,
                    )
                nc.scalar.copy(r16[:, c * R_DIM:(c + 1) * R_DIM], r_ps[:])
                cy_ps = ps_cy.tile([SUB, CY_DIM], f32, tag="cy", space="PSUM")
                nc.tensor.matmul(
                    cy_ps[:], rsh_t[:, esl], cc2_s[:], start=True, stop=True
                )
                nc.scalar.copy(cy16[:, c * CY_DIM:(c + 1) * CY_DIM], cy_ps[:])

            # ---- D-stage ----
            # D2_i [p, c, o, m] tiles (m = (j,v,k), contiguous)
            d2 = [
                d2pool.tile([SUB, C * _no(i) * M_I[i]], f16, tag=f"d2_{i}",
                            name=f"d2_{i}")
                for i in range(3)
            ]
            # j=0: direct product into D2_i[:, :, :, 0:8]
            for i in range(3):
                no, m = _no(i), M_I[i]
                out_ap = (
                    d2[i][:].rearrange("p (c o m) -> p c o m", c=C, o=no)[:, :, :, 0:8]
                )
                f_ap = (
                    fg_t[:].rearrange("p (c f) -> p c f", c=C)[:, :, 0:8]
                    .unsqueeze(2).broadcast_to((SUB, C, no, 8))
                )
                cyb = JOFF2[0] + _woff(0, i)
                cy_ap = (
                    cy16[:].rearrange("p (c w) -> p c w", c=C)[:, :, cyb:cyb + no]
                    .unsqueeze(3).broadcast_to((SUB, C, no, 8))
                )
                nc.vector.tensor_tensor(out_ap, f_ap, cy_ap, OP.mult)
            # j=2: product on DVE, ii-reduction on PE (PSUM accumulate)
            j2 = 2
            nj2, wj2 = _nj(j2), W_J[j2]
            dt2 = wpool.tile([SUB, C * 8 * wj2 * nj2], f16, tag="dt2w")
            dt2v = dt2[:].rearrange("p (c v w i) -> p c v w i", c=C, v=8, w=wj2)
            for c in range(C):
                f_ap = (
                    fg_t[:, c * FEAT_DIM + FEAT_OFF[j2]:
                         c * FEAT_DIM + FEAT_OFF[j2 + 1]]
                    .rearrange("p (v i) -> p v i", v=8)
                    .unsqueeze(2).broadcast_to((SUB, 8, wj2, nj2))
                )
                cy_ap = (
                    cy16[:, c * CY_DIM + JOFF2[j2]:
                         c * CY_DIM + JOFF2[j2 + 1]]
                    .rearrange("p (w i) -> p w i", w=wj2)
                    .unsqueeze(1).broadcast_to((SUB, 8, wj2, nj2))
                )
                nc.vector.tensor_tensor(dt2v[:, c], f_ap, cy_ap, OP.mult)
            dps = ps_dred.tile([SUB, 1024], f32, tag="dred", space="PSUM")
            dps1 = dps
            nwc = 8 * wj2
            for c in range(C):
                bank = (c % 2) * 512
                for ii in range(nj2):
                    nc.tensor.matmul(
                        dps[:, bank:bank + nwc], ident_s[:],
                        dt2v[:, c, :, :, ii],
                        start=(ii == 0), stop=(ii == nj2 - 1),
                    )
                dv = dps[:, bank:bank + nwc].rearrange(
                    "p (v w) -> p v w", v=8)
                for i in range(3):
                    nl, no, m = _nl(i, j2), _no(i), M_I[i]
                    in_ap = (
                        dv[:, :, _woff(j2, i):_woff(j2, i) + no * nl]
                        .rearrange("p v (o k) -> p o v k", o=no)
                    )
                    out_ap = (
                        d2[i][:].rearrange("p (c o m) -> p c o m", c=C, o=no)
                        [:, c, :, _moff(i, j2):_moff(i, j2) + 8 * nl]
                        .rearrange("p o (v k) -> p o v k", v=8)
                    )
                    nc.scalar.copy(out_ap, in_ap)

            # j=1: product on DVE, ii-reduction on PE (PSUM accumulate)
            j1 = 1
            nj1, wj1 = _nj(j1), W_J[j1]
            dt1 = wpool.tile([SUB, C * 8 * wj1 * nj1], f16, tag="dt1w")
            dt1v = dt1[:].rearrange("p (c v w i) -> p c v w i", c=C, v=8, w=wj1)
            for c in range(C):
                f_ap = (
                    fg_t[:, c * FEAT_DIM + FEAT_OFF[j1]:
                         c * FEAT_DIM + FEAT_OFF[j1 + 1]]
                    .rearrange("p (v i) -> p v i", v=8)
                    .unsqueeze(2).broadcast_to((SUB, 8, wj1, nj1))
                )
                cy_ap = (
                    cy16[:, c * CY_DIM + JOFF2[j1]:
                         c * CY_DIM + JOFF2[j1 + 1]]
                    .rearrange("p (w i) -> p w i", w=wj1)
                    .unsqueeze(1).broadcast_to((SUB, 8, wj1, nj1))
                )
                nc.vector.tensor_tensor(dt1v[:, c], f_ap, cy_ap, OP.mult)
            nwc1 = 8 * wj1
            for c in range(C):
                bank = (c % 2) * 512
                for ii in range(nj1):
                    nc.tensor.matmul(
                        dps1[:, bank:bank + nwc1], ident_s[:],
                        dt1v[:, c, :, :, ii],
                        start=(ii == 0), stop=(ii == nj1 - 1),
                    )
                dv1 = dps1[:, bank:bank + nwc1].rearrange(
                    "p (v w) -> p v w", v=8)
                for i in range(3):
                    nl, no, m = _nl(i, j1), _no(i), M_I[i]
                    in_ap = (
                        dv1[:, :, _woff(j1, i):_woff(j1, i) + no * nl]
                        .rearrange("p v (o k) -> p o v k", o=no)
                    )
                    out_ap = (
                        d2[i][:].rearrange("p (c o m) -> p c o m", c=C, o=no)
                        [:, c, :, _moff(i, j1):_moff(i, j1) + 8 * nl]
                        .rearrange("p o (v k) -> p o v k", v=8)
                    )
                    nc.scalar.copy(out_ap, in_ap)

            # ---- R-stage products: rt_i [p, c, u, o, m], full-m per (c, i) ----
            rt = [
                wpool.tile([SUB, C * 8 * _no(i) * M_I[i]], f16, tag=f"rt{i}",
                           name=f"rt{i}")
                for i in range(3)
            ]
            for i in range(3):
                no, m = _no(i), M_I[i]
                rt4 = rt[i][:].rearrange(
                    "p (c u o m) -> p c u o m", c=C, u=8, o=no)
                for c in range(C):
                    r_ap = (
                        r16[:, c * R_DIM + IOFF[i]:
                            c * R_DIM + IOFF[i] + 8 * m]
                        .rearrange("p (u m) -> p u m", u=8)
                        .unsqueeze(2).broadcast_to((SUB, 8, no, m))
                    )
                    d_ap = (
                        d2[i][:, c * no * m:(c + 1) * no * m]
                        .rearrange("p (o m) -> p o m", o=no)
                        .unsqueeze(1).broadcast_to((SUB, 8, no, m))
                    )
                    nc.vector.tensor_tensor(rt4[:, c], r_ap, d_ap, OP.mult)

            # ---- R-stage reduction: halving tree + TR tail -> msg ----
            msg_t = mpool.tile([SUB, C * FEAT_DIM], f16, tag="msg")
            msg4 = msg_t[:].rearrange("p (c f) -> p c f", c=C)
            for i in range(3):
                no, m = _no(i), M_I[i]
                g = C * 8 * no
                cur = rt[i][:].rearrange("p (g m) -> p g m", g=g)
                width = m
                while width > 9:
                    half = width // 2
                    t = wpool.tile([SUB, g * half], f16, tag=f"tr{i}_{width}")
                    tv = t[:].rearrange("p (g m) -> p g m", g=g)
                    nc.vector.tensor_tensor(
                        tv, cur[:, :, 0:half], cur[:, :, half:width], OP.add)
                    cur, width = tv, half
                in_ap = cur[:, :, 0:width].rearrange(
                    "p (c g) m -> p c g m", c=C)
                out_ap = msg4[:, :, FEAT_OFF[i]:FEAT_OFF[i + 1]]
                nc.vector.tensor_reduce(
                    out_ap, in_ap, mybir.AxisListType.X, OP.add)

            # ---- combine duplicate dsts (host sel), then scatter ----
            msg2_t = mpool.tile([SUB, C * FEAT_DIM], f16, tag="msg2")
            for c in range(C):
                cmb_ps = ps_cmb.tile([SUB, FEAT_DIM], f32, tag=f"cmb{c % 2}",
                                     space="PSUM")
                nc.tensor.matmul(
                    cmb_ps[:], sel_t[:, c * SUB:(c + 1) * SUB],
                    msg_t[:, c * FEAT_DIM:(c + 1) * FEAT_DIM],
                    start=True, stop=True,
                )
                nc.scalar.copy(
                    msg2_t[:, c * FEAT_DIM:(c + 1) * FEAT_DIM], cmb_ps[:])
            for c in range(C):
                nc.gpsimd.indirect_dma_start(
                    out=out_d[:],
                    out_offset=bass.IndirectOffsetOnAxis(
                        ap=dst_t[:, c:c + 1], axis=0),
                    in_=msg2_t[:, c * FEAT_DIM:(c + 1) * FEAT_DIM],
                    in_offset=None,
                )
            if debug and s == 0:
                for name, t in (("dbg_fg", fg_t), ("dbg_r16", r16),
                                ("dbg_cy16", cy16), ("dbg_d2_0", d2[0]),
                                ("dbg_d2_1", d2[1]), ("dbg_d2_2", d2[2]),
                                ("dbg_rt_2", rt[2]), ("dbg_msg", msg_t)):
                    nc.sync.dma_start(dbg_d[name][:], t[:])
                hd = hpool.tile([H, SUPER], f16, tag="hdump")
                nc.scalar.copy(hd[:], h[:])
                nc.sync.dma_start(dbg_d["dbg_h"][:], hd[:])

    nc.finalize()
    return nc


# ----------------- host side -----------------

def _prep_consts(cc, W0, W1, W2, W3):
    W0p, W1p, W2p, W3p = fold_weights(W0, W1, W2, W3)
    cc2 = build_cc2(np.asarray(cc, dtype=np.float32)).astype(np.float16)
    centers = np.linspace(MIN_R, MAX_R, N_BASIS).astype(np.float32)
    spacing = (MAX_R - MIN_R) / (N_BASIS - 1)
    cscale = np.full((N_BASIS, 1), 1.0 / spacing, dtype=np.float32)
    cbias = (-centers / spacing).astype(np.float32).reshape(N_BASIS, 1)
    return W0p, W1p, W2p, W3p, cc2, cscale, cbias


def pack_edges(dst: np.ndarray, n_nodes: int):
    """Group edges by dst, bin-pack groups into 128-edge tiles (best-fit
    decreasing). Returns int64 [n_tiles, SUB] of edge ids, -1 for pads."""
    import bisect

    order = np.argsort(dst, kind="stable")
    ds = dst[order]
    starts = np.flatnonzero(np.r_[True, ds[1:] != ds[:-1]])
    ends = np.r_[starts[1:], len(ds)]
    runs = sorted(
        ((int(e - s), int(s), int(e)) for s, e in zip(starts, ends)),
        key=lambda r: -r[0],
    )
    assert runs[0][0] <= SUB, f"node with {runs[0][0]} > {SUB} in-edges"
    bins, rems, binidx = [], [], []
    for L, s, e in runs:
        k = bisect.bisect_left(rems, L)
        if k == len(rems):
            bins.append([(s, e)])
            r, bi = SUB - L, len(bins) - 1
        else:
            bi = binidx[k]
            r = rems[k] - L
            del rems[k], binidx[k]
            bins[bi].append((s, e))
        j = bisect.bisect_left(rems, r)
        rems.insert(j, r)
        binidx.insert(j, bi)
    tiles = []
    for b in bins:
        cur = []
        for s, e in b:
            cur.extend(order[s:e].tolist())
        cur.extend([-1] * (SUB - len(cur)))
        tiles.append(cur)
    return np.array(tiles, dtype=np.int64)


def _build_and_maps(edge_index, features, radii, rsh, cc, W0, W1, W2, W3):
    edge_index = np.asarray(edge_index)
    features = np.asarray(features, dtype=np.float32)
    radii = np.asarray(radii, dtype=np.float32)
    rsh = np.asarray(rsh, dtype=np.float32)
    n_nodes = features.shape[0]
    E = radii.shape[0]

    W0p, W1p, W2p, W3p, cc2, cscale, cbias = _prep_consts(cc, W0, W1, W2, W3)
    feat16 = features.astype(np.float16)

    src = edge_index[0].astype(np.int64)
    dst = edge_index[1].astype(np.int64)
    order = np.argsort(dst, kind="stable")

    epc = -(-E // N_CORES)
    n_super = -(-epc // SUPER)
    e_pad = n_super * SUPER

    nc = build_program(n_super, n_nodes)

    rshT = np.ascontiguousarray(rsh.T).astype(np.float16)
    in_maps = []
    for kcore in range(N_CORES):
        ids = order[kcore * epc:(kcore + 1) * epc]
        nvalid = ids.size
        idx = np.concatenate([ids, np.zeros(e_pad - nvalid, np.int64)])
        valid = np.arange(e_pad) < nvalid

        rshT_s = np.where(valid[None, :], rshT[:, idx], np.float16(0.0))
        radii_s = np.where(valid, radii[idx], np.float32(1.0)).reshape(1, -1)
        src_s = np.where(valid, src[idx], 0).astype(np.int32)
        src_r = src_s.reshape(n_super, N_SUB, SUB).transpose(0, 2, 1)
        in_maps.append(
            dict(
                rshT=np.ascontiguousarray(rshT_s),
                radii=np.ascontiguousarray(radii_s),
                srcidx=np.ascontiguousarray(src_r),
                features=feat16,
                W0p=W0p, W1p=W1p, W2p=W2p, W3p=W3p,
                CC2=cc2, cscale=cscale, cbias=cbias,
            )
        )

    post = dict(order=order, dst=dst, n_nodes=n_nodes, E=E,
                epc=epc, n_super=n_super, e_pad=e_pad)
    return nc, in_maps, post


def _segment_sum(res_list, post):
    """Host-side: unpack per-core dense messages, segment-sum by dst."""
    E, epc, e_pad = post["E"], post["epc"], post["e_pad"]
    n_super = post["n_super"]
    msgs = np.empty((E, FEAT_DIM), dtype=np.float32)
    for k, r in enumerate(res_list):
        m = np.asarray(r["out"])           # [n_super, SUB, N_SUB*FEAT]
        m = (m.reshape(n_super, SUB, N_SUB, FEAT_DIM)
             .transpose(0, 2, 1, 3).reshape(e_pad, FEAT_DIM))
        lo = k * epc
        hi = min(lo + epc, E)
        msgs[lo:hi] = m[: hi - lo]
    ds = post["dst"][post["order"]]
    starts = np.flatnonzero(np.r_[True, ds[1:] != ds[:-1]])
    sums = np.add.reduceat(msgs, starts, axis=0)
    out = np.zeros((post["n_nodes"], FEAT_DIM), dtype=np.float32)
    out[ds[starts]] = sums
    return out


def kernel(edge_index, features, radii, rsh, cc, W0, W1, W2, W3):
    from concourse.bass_utils import run_bass_kernel_spmd

    nc, in_maps, post = _build_and_maps(
        edge_index, features, radii, rsh, cc, W0, W1, W2, W3
    )
    res = run_bass_kernel_spmd(nc, in_maps, core_ids=list(range(N_CORES)))
    return _segment_sum(res.results, post)


def _install_ntff_shim():
    """Provide antenv.axon_hooks + the ctypes NTFF hook if absent."""
    import contextlib
    import ctypes
    import sys
    import types

    try:
        from antenv.axon_hooks import get_axon_ntff_profile_hook  # noqa: F401
        return
    except ImportError:
        pass

    holder = {}
    mod = types.ModuleType("antenv.axon_hooks")
    mod.set_axon_ntff_profile_hook = lambda h: holder.__setitem__("h", h)
    mod.get_axon_ntff_profile_hook = lambda: holder.get("h")
    import antenv

    sys.modules["antenv.axon_hooks"] = mod
    antenv.axon_hooks = mod

    so_path = "/opt/axon/libaxon_pjrt.so"
    try:
        lib = ctypes.CDLL(so_path)
    except OSError:
        return
    if not hasattr(lib, "axon_start_nrt_profile"):
        return
    lib.axon_start_nrt_profile.argtypes = [
        ctypes.POINTER(ctypes.c_int64),
        ctypes.c_size_t,
    ]
    lib.axon_start_nrt_profile.restype = ctypes.c_int64
    lib.axon_stop_nrt_profile.argtypes = [ctypes.c_char_p]
    lib.axon_stop_nrt_profile.restype = ctypes.c_int64

    @contextlib.contextmanager
    def _hook(output_dir, device_ids):
        import jax

        jax.devices()
        if device_ids:
            ids = (ctypes.c_int64 * len(device_ids))(*device_ids)
            rc = lib.axon_start_nrt_profile(ids, len(device_ids))
        else:
            rc = lib.axon_start_nrt_profile(None, 0)
        if rc != 0:
            raise RuntimeError(f"axon_start_nrt_profile rc={rc}")
        try:
            yield
        finally:
            n = lib.axon_stop_nrt_profile(str(output_dir).encode())
            print(f"ntff profile: {n} file(s) written to {output_dir}")

    mod.set_axon_ntff_profile_hook(_hook)


def kernel_traced(edge_index, features, radii, rsh, cc, W0, W1, W2, W3,
                  trace_cores=None, tmpdir=None):
    """Run with NTFF tracing; returns BassKernelResults."""
    _install_ntff_shim()
    from concourse import bass_utils

    bass_utils.upload_artifacts = lambda d: f"local:{d}"

    nc, in_maps, post = _build_and_maps(
        edge_index, features, radii, rsh, cc, W0, W1, W2, W3
    )
    return bass_utils.run_bass_kernel_spmd(
        nc, in_maps, core_ids=list(range(N_CORES)), trace=True,
        trace_cores=trace_cores, tmpdir=tmpdir,
    )


# revision 27
# speedup vs baseline: 1.1003x; 1.1003x over previous
"""Trainium2 Bass kernel for nn_MinimalNetwork (equivariant GNN message passing).

v2: fp16 pipeline tuned for DVE 2x perf mode.

Sharded over 8 NeuronCores by edge (data parallel). Per core, per
512-edge super-tile (4 sub-tiles of 128 edges = partition dim):
  radial basis -> 3-layer silu MLP (fp16 matmuls, PE)
  R   = h^T @ W3perm   [128, 1216] fp16 (PE + copy)
  CY  = rsh @ CC2perm  [128, 259]  fp16 (PE + copy)
  D-stage: per-edge F x CY products (DVE 2x) + ii-reduction (DVE)
  R-stage: per-edge R x D products (DVE 2x) + m-reduction halving tree
  combine: sel @ msg matmul (host-built per-tile dedup matrices, PE)
  scatter: one 4-column indirect DMA per super into fp16 node table.
Host sums the 8 per-core tables in fp32.

Self-contained: shapes/layout hardcoded for the 200000-edge/12500-node
instance; works for any edge count that bin-packs into 128-edge tiles.
"""

import math
from contextlib import ExitStack
from itertools import accumulate

import numpy as np

# ----------------- problem constants (hardcoded) -----------------
N_NODES = 12500
N_EDGES = 200000
N_CORES = 8
LO = [0, 1, 2]            # l value per block (multiplicity 8 each)
SH_DIM = 25
FEAT_OFF = [0, 8, 32, 72]
FEAT_DIM = 72
N_BASIS, H = 10, 100
R_DIM = 1216
MIN_R, MAX_R = 0.7, 3.2
SWISH_SCALE = 1.679177
SUB = 128
SUPER = 512
N_SUB = SUPER // SUB


def _nl(i, j):
    return 2 * min(i, j) + 1


def _no(i):
    return 2 * i + 1


def _nj(j):
    return 2 * j + 1


M_I = [sum(8 * _nl(i, j) for j in range(3)) for i in range(3)]      # 24,56,72
IOFF = [0] + list(accumulate(8 * m for m in M_I))                    # R col blocks


def _moff(i, j):
    return sum(8 * _nl(i, j2) for j2 in range(j))


W_J = [sum(_no(i) * _nl(i, j) for i in range(3)) for j in range(3)]  # 9,25,35


def _woff(j, i):
    return sum(_no(i2) * _nl(i2, j) for i2 in range(i))


JOFF2 = [0] + list(accumulate(W_J[j] * _nj(j) for j in range(3)))    # cc2 col blocks
CY_DIM = JOFF2[-1]                                                   # 259

# original (reference) R block offsets: pairs (i,j) blocks of [u][v][k]
R_OFF = [0] + list(
    accumulate(8 * 8 * _nl(i, j) for i in range(3) for j in range(3))
)


def _cc_layout():
    layout, off = {}, 0
    for lo in LO:
        for li in LO:
            for lf in range(abs(lo - li), lo + li + 1):
                if (lo, li, lf) not in layout:
                    shp = (2 * lo + 1, 2 * li + 1, 2 * lf + 1)
                    layout[(lo, li, lf)] = (off, shp)
                    off += shp[0] * shp[1] * shp[2]
    return layout, off


CC_LAYOUT, CC_TOTAL = _cc_layout()  # 1225


def _norm_coef():
    nc = np.zeros((3, 3), dtype=np.float64)
    for i, lo in enumerate(LO):
        ns = sum(8 * (2 * min(lo, li) + 1) for li in LO)
        nc[i, :] = math.sqrt(4 * math.pi) * math.sqrt(2 * lo + 1) / math.sqrt(ns)
    return nc


NORM = _norm_coef()


def build_cc2(cc: np.ndarray) -> np.ndarray:
    """CC2 [25, CY_DIM]; column = JOFF2[j] + w*nj + ii, w = woff(j,i)+o*nl+k."""
    cc2 = np.zeros((SH_DIM, CY_DIM), dtype=np.float32)
    for j in range(3):
        nj = _nj(j)
        for i in range(3):
            nl, no = _nl(i, j), _no(i)
            for k, lf in enumerate(range(abs(i - j), i + j + 1)):
                if k >= nl:
                    break
                off, shp = CC_LAYOUT[(i, j, lf)]
                C = cc[off: off + shp[0] * shp[1] * shp[2]].reshape(shp)
                for o in range(no):
                    w = _woff(j, i) + o * nl + k
                    for ii in range(nj):
                        col = JOFF2[j] + w * nj + ii
                        cc2[lf * lf: lf * lf + 2 * lf + 1, col] = (
                            np.float32(NORM[i, j]) * C[o, ii, :]
                        )
    return cc2


def permute_w3(W3s: np.ndarray) -> np.ndarray:
    """Reorder W3 columns: newcol(i,u,j,v,k) = IOFF[i]+u*M_I[i]+moff(i,j)+v*nl+k."""
    perm = np.zeros(R_DIM, dtype=np.int64)
    for i in range(3):
        for j in range(3):
            nl = _nl(i, j)
            p = i * 3 + j
            for u in range(8):
                for v in range(8):
                    for k in range(nl):
                        old = R_OFF[p] + u * (8 * nl) + v * nl + k
                        new = IOFF[i] + u * M_I[i] + _moff(i, j) + v * nl + k
                        perm[new] = old
    return np.ascontiguousarray(W3s[:, perm])


def fold_weights(W0, W1, W2, W3):
    s = SWISH_SCALE
    return (
        (W0 / math.sqrt(N_BASIS)).astype(np.float16),
        (s * W1 / math.sqrt(H)).astype(np.float16),
        (s * W2 / math.sqrt(H)).astype(np.float16),
        permute_w3((s * W3 / math.sqrt(H)).astype(np.float32)).astype(np.float16),
    )


# ----------------- bass program -----------------

def build_program(n_super: int, n_nodes: int, debug: bool = False):
    import concourse.bass as bass
    import concourse.tile as tile
    from concourse import bacc, mybir

    f32 = mybir.dt.float32
    f16 = mybir.dt.float16
    i32 = mybir.dt.int32
    AF = mybir.ActivationFunctionType
    OP = mybir.AluOpType

    nc = bacc.Bacc()

    e_pad = n_super * SUPER
    n_out = n_nodes + 1

    rshT_d = nc.dram_tensor("rshT", [SH_DIM, e_pad], f16, kind="ExternalInput")
    radii_d = nc.dram_tensor("radii", [1, e_pad], f32, kind="ExternalInput")
    src_d = nc.dram_tensor("srcidx", [n_super, SUB, N_SUB], i32, kind="ExternalInput")
    dst_d = nc.dram_tensor("dstidx", [n_super, SUB, N_SUB], i32, kind="ExternalInput")
    sel_d = nc.dram_tensor("sel", [n_super, SUB, N_SUB * SUB], f16, kind="ExternalInput")
    feat_d = nc.dram_tensor("features", [n_out, FEAT_DIM], f16, kind="ExternalInput")
    w0_d = nc.dram_tensor("W0p", [N_BASIS, H], f16, kind="ExternalInput")
    w1_d = nc.dram_tensor("W1p", [H, H], f16, kind="ExternalInput")
    w2_d = nc.dram_tensor("W2p", [H, H], f16, kind="ExternalInput")
    w3_d = nc.dram_tensor("W3p", [H, R_DIM], f16, kind="ExternalInput")
    cc2_d = nc.dram_tensor("CC2", [SH_DIM, CY_DIM], f16, kind="ExternalInput")
    csc_d = nc.dram_tensor("cscale", [N_BASIS, 1], f32, kind="ExternalInput")
    cbi_d = nc.dram_tensor("cbias", [N_BASIS, 1], f32, kind="ExternalInput")
    out_d = nc.dram_tensor("out", [n_out, FEAT_DIM], f16, kind="ExternalOutput")
    if debug:
        dbg = {
            "dbg_fg": [SUB, N_SUB * FEAT_DIM],
            "dbg_r16": [SUB, N_SUB * R_DIM],
            "dbg_cy16": [SUB, N_SUB * CY_DIM],
            "dbg_d2_0": [SUB, N_SUB * 1 * 24],
            "dbg_d2_1": [SUB, N_SUB * 3 * 56],
            "dbg_d2_2": [SUB, N_SUB * 5 * 72],
            "dbg_rt_2": [SUB, N_SUB * 8 * 5 * 72],
            "dbg_msg": [SUB, N_SUB * FEAT_DIM],
            "dbg_msg2": [SUB, N_SUB * FEAT_DIM],
            "dbg_h": [H, SUPER],
        }
        dbg_d = {k: nc.dram_tensor(k, v, f16, kind="ExternalOutput")
                 for k, v in dbg.items()}

    with tile.TileContext(nc) as tc, ExitStack() as ctx:
        cpool = ctx.enter_context(tc.tile_pool(name="consts", bufs=1))
        inpool = ctx.enter_context(tc.tile_pool(name="in", bufs=3))
        hpool = ctx.enter_context(tc.tile_pool(name="h", bufs=2))
        dpool = ctx.enter_context(tc.tile_pool(name="xfer", bufs=2))
        wpool = ctx.enter_context(tc.tile_pool(name="work", bufs=1))
        mpool = ctx.enter_context(tc.tile_pool(name="msg", bufs=2))
        ps_mlp = ctx.enter_context(tc.tile_pool(name="psmlp", bufs=1, space="PSUM"))
        ps_r = ctx.enter_context(tc.tile_pool(name="psr", bufs=1, space="PSUM"))
        ps_cy = ctx.enter_context(tc.tile_pool(name="pscy", bufs=1, space="PSUM"))
        ps_dred = ctx.enter_context(tc.tile_pool(name="psdred", bufs=1, space="PSUM"))

        w0_s = cpool.tile([N_BASIS, H], f16)
        w1_s = cpool.tile([H, H], f16)
        w2_s = cpool.tile([H, H], f16)
        w3_s = cpool.tile([H, R_DIM], f16)
        cc2_s = cpool.tile([SH_DIM, CY_DIM], f16)
        csc_s = cpool.tile([N_BASIS, 1], f32)
        cbi_s = cpool.tile([N_BASIS, 1], f32)
        ones_s = cpool.tile([1, N_BASIS], f32)
        ident_s = cpool.tile([SUB, SUB], f16)
        from concourse.masks import make_identity
        make_identity(nc, ident_s[:])
        for t, d in (
            (w0_s, w0_d), (w1_s, w1_d), (w2_s, w2_d), (w3_s, w3_d),
            (cc2_s, cc2_d), (csc_s, csc_d), (cbi_s, cbi_d),
        ):
            nc.sync.dma_start(t[:], d[:])
        nc.vector.memset(ones_s[:], 1.0)

        # zero-init output table
        nfull = n_out // SUB
        nc.sync.dma_start(
            out_d[: nfull * SUB, :].rearrange("(a p) c -> p a c", p=SUB),
            zero_s[:].unsqueeze(1).broadcast_to((SUB, nfull, FEAT_DIM)),
        )
        rem = n_out - nfull * SUB
        if rem:
            nc.sync.dma_start(out_d[nfull * SUB:, :], zero_s[:rem, :])

        C = N_SUB  # 4 sub-tiles folded per instruction

        lp = ctx.enter_context(
            nc.allow_low_precision(reason="fp16 pipeline within 2e-2 tolerance"))
        for s in range(n_super):
            e0 = s * SUPER
            # ---- loads ----
            rsh_t = inpool.tile([SH_DIM, SUPER], f16, tag="rsh")
            nc.sync.dma_start(rsh_t[:], rshT_d[:, e0:e0 + SUPER])
            rad_t = inpool.tile([1, SUPER], f32, tag="rad")
            nc.sync.dma_start(rad_t[:], radii_d[:, e0:e0 + SUPER])
            src_t = inpool.tile([SUB, C], i32, tag="src")
            nc.sync.dma_start(src_t[:], src_d[s])
            dst_t = inpool.tile([SUB, C], i32, tag="dst")
            nc.sync.dma_start(dst_t[:], dst_d[s])
            sel_t = inpool.tile([SUB, C * SUB], f16, tag="sel")
            nc.sync.dma_start(sel_t[:], sel_d[s])
            fg_t = inpool.tile([SUB, C * FEAT_DIM], f16, tag="fg")
            for c in range(C):
                nc.gpsimd.indirect_dma_start(
                    out=fg_t[:, c * FEAT_DIM:(c + 1) * FEAT_DIM],
                    out_offset=None,
                    in_=feat_d[:],
                    in_offset=bass.IndirectOffsetOnAxis(
                        ap=src_t[:, c:c + 1], axis=0),
                )

            # ---- radial basis (fp32 -> fp16), reusing the MLP PSUM bank ----
            rb_full = ps_mlp.tile([H, SUPER], f32, tag="hp", space="PSUM",
                                  name="rb")
            rb_ps = rb_full[0:N_BASIS, :]
            nc.tensor.matmul(rb_ps, ones_s[:], rad_t[:], start=True, stop=True)
            z2_t = hpool.tile([N_BASIS, SUPER], f32, tag="z2")
            nc.scalar.activation(
                z2_t[:], rb_ps, AF.Square, bias=cbi_s[:], scale=csc_s[:]
            )
            bas_t = hpool.tile([N_BASIS, SUPER], f16, tag="bas")
            nc.scalar.activation(bas_t[:], z2_t[:], AF.Exp, scale=-1.0)

            # ---- MLP (fp16 matmuls, silu out fp16) ----
            h = bas_t
            for li, w_s in enumerate((w0_s, w1_s, w2_s)):
                hp = ps_mlp.tile([H, SUPER], f32, tag="hp", space="PSUM")
                nc.tensor.matmul(hp[:], w_s[:], h[:], start=True, stop=True)
                hn = hpool.tile([H, SUPER], f16, tag=f"h{li}")
                nc.scalar.activation(hn[:], hp[:], AF.Silu)
                h = hn

            # ---- R and CY per sub-tile ----
            r16 = dpool.tile([SUB, C * R_DIM], f16, tag="r16")
            cy16 = dpool.tile([SUB, C * CY_DIM], f16, tag="cy16")
            for c in range(C):
                esl = slice(c * SUB, (c + 1) * SUB)
                r_ps = ps_r.tile([SUB, R_DIM], f32, tag="r", space="PSUM")
                for c0 in range(0, R_DIM, 512):
                    c1 = min(c0 + 512, R_DIM)
                    nc.tensor.matmul(
                        r_ps[:, c0:c1], h[:, esl], w3_s[:, c0:c1],
                        start=True, stop=True,
                    )
                nc.scalar.copy(r16[:, c * R_DIM:(c + 1) * R_DIM], r_ps[:])
                cy_ps = ps_cy.tile([SUB, CY_DIM], f32, tag="cy", space="PSUM")
                nc.tensor.matmul(
                    cy_ps[:], rsh_t[:, esl], cc2_s[:], start=True, stop=True
                )
                nc.scalar.copy(cy16[:, c * CY_DIM:(c + 1) * CY_DIM], cy_ps[:])

            # ---- D-stage ----
            # D2_i [p, c, o, m] tiles (m = (j,v,k), contiguous)
            d2 = [
                wpool.tile([SUB, C * _no(i) * M_I[i]], f16, tag=f"d2_{i}",
                           name=f"d2_{i}")
                for i in range(3)
            ]
            # j=0: direct product into D2_i[:, :, :, 0:8]
            for i in range(3):
                no, m = _no(i), M_I[i]
                out_ap = (
                    d2[i][:].rearrange("p (c o m) -> p c o m", c=C, o=no)[:, :, :, 0:8]
                )
                f_ap = (
                    fg_t[:].rearrange("p (c f) -> p c f", c=C)[:, :, 0:8]
                    .unsqueeze(2).broadcast_to((SUB, C, no, 8))
                )
                cyb = JOFF2[0] + _woff(0, i)
                cy_ap = (
                    cy16[:].rearrange("p (c w) -> p c w", c=C)[:, :, cyb:cyb + no]
                    .unsqueeze(3).broadcast_to((SUB, C, no, 8))
                )
                nc.vector.tensor_tensor(out_ap, f_ap, cy_ap, OP.mult)
            # j=2: product on DVE, ii-reduction on PE (PSUM accumulate)
            j2 = 2
            nj2, wj2 = _nj(j2), W_J[j2]
            dt2 = wpool.tile([SUB, C * 8 * wj2 * nj2], f16, tag="dt2w")
            dt2v = dt2[:].rearrange("p (c v w i) -> p c v w i", c=C, v=8, w=wj2)
            for c in range(C):
                f_ap = (
                    fg_t[:, c * FEAT_DIM + FEAT_OFF[j2]:
                         c * FEAT_DIM + FEAT_OFF[j2 + 1]]
                    .rearrange("p (v i) -> p v i", v=8)
                    .unsqueeze(2).broadcast_to((SUB, 8, wj2, nj2))
                )
                cy_ap = (
                    cy16[:, c * CY_DIM + JOFF2[j2]:
                         c * CY_DIM + JOFF2[j2 + 1]]
                    .rearrange("p (w i) -> p w i", w=wj2)
                    .unsqueeze(1).broadcast_to((SUB, 8, wj2, nj2))
                )
                nc.vector.tensor_tensor(dt2v[:, c], f_ap, cy_ap, OP.mult)
            dps = ps_dred.tile([SUB, 1024], f32, tag="dred", space="PSUM")
            dps1 = dps
            nwc = 8 * wj2
            for c in range(C):
                bank = (c % 2) * 512
                for ii in range(nj2):
                    nc.tensor.matmul(
                        dps[:, bank:bank + nwc], ident_s[:],
                        dt2v[:, c, :, :, ii],
                        start=(ii == 0), stop=(ii == nj2 - 1),
                    )
                dv = dps[:, bank:bank + nwc].rearrange(
                    "p (v w) -> p v w", v=8)
                for i in range(3):
                    nl, no, m = _nl(i, j2), _no(i), M_I[i]
                    in_ap = (
                        dv[:, :, _woff(j2, i):_woff(j2, i) + no * nl]
                        .rearrange("p v (o k) -> p o v k", o=no)
                    )
                    out_ap = (
                        d2[i][:].rearrange("p (c o m) -> p c o m", c=C, o=no)
                        [:, c, :, _moff(i, j2):_moff(i, j2) + 8 * nl]
                        .rearrange("p o (v k) -> p o v k", v=8)
                    )
                    nc.scalar.copy(out_ap, in_ap)

            # j=1: product on DVE, ii-reduction on PE (PSUM accumulate)
            j1 = 1
            nj1, wj1 = _nj(j1), W_J[j1]
            dt1 = wpool.tile([SUB, C * 8 * wj1 * nj1], f16, tag="dt1w")
            dt1v = dt1[:].rearrange("p (c v w i) -> p c v w i", c=C, v=8, w=wj1)
            for c in range(C):
                f_ap = (
                    fg_t[:, c * FEAT_DIM + FEAT_OFF[j1]:
                         c * FEAT_DIM + FEAT_OFF[j1 + 1]]
                    .rearrange("p (v i) -> p v i", v=8)
                    .unsqueeze(2).broadcast_to((SUB, 8, wj1, nj1))
                )
                cy_ap = (
                    cy16[:, c * CY_DIM + JOFF2[j1]:
                         c * CY_DIM + JOFF2[j1 + 1]]
                    .rearrange("p (w i) -> p w i", w=wj1)
                    .unsqueeze(1).broadcast_to((SUB, 8, wj1, nj1))
                )
                nc.vector.tensor_tensor(dt1v[:, c], f_ap, cy_ap, OP.mult)
            nwc1 = 8 * wj1
            for c in range(C):
                bank = (c % 2) * 512
                for ii in range(nj1):
                    nc.tensor.matmul(
                        dps1[:, bank:bank + nwc1], ident_s[:],
                        dt1v[:, c, :, :, ii],
                        start=(ii == 0), stop=(ii == nj1 - 1),
                    )
                dv1 = dps1[:, bank:bank + nwc1].rearrange(
                    "p (v w) -> p v w", v=8)
                for i in range(3):
                    nl, no, m = _nl(i, j1), _no(i), M_I[i]
                    in_ap = (
                        dv1[:, :, _woff(j1, i):_woff(j1, i) + no * nl]
                        .rearrange("p v (o k) -> p o v k", o=no)
                    )
                    out_ap = (
                        d2[i][:].rearrange("p (c o m) -> p c o m", c=C, o=no)
                        [:, c, :, _moff(i, j1):_moff(i, j1) + 8 * nl]
                        .rearrange("p o (v k) -> p o v k", v=8)
                    )
                    nc.scalar.copy(out_ap, in_ap)

            # ---- R-stage products: rt_i [p, c, u, o, m], full-m per (c, i) ----
            rt = [
                wpool.tile([SUB, C * 8 * _no(i) * M_I[i]], f16, tag=f"rt{i}",
                           name=f"rt{i}")
                for i in range(3)
            ]
            for i in range(3):
                no, m = _no(i), M_I[i]
                rt4 = rt[i][:].rearrange(
                    "p (c u o m) -> p c u o m", c=C, u=8, o=no)
                for c in range(C):
                    r_ap = (
                        r16[:, c * R_DIM + IOFF[i]:
                            c * R_DIM + IOFF[i] + 8 * m]
                        .rearrange("p (u m) -> p u m", u=8)
                        .unsqueeze(2).broadcast_to((SUB, 8, no, m))
                    )
                    d_ap = (
                        d2[i][:, c * no * m:(c + 1) * no * m]
                        .rearrange("p (o m) -> p o m", o=no)
                        .unsqueeze(1).broadcast_to((SUB, 8, no, m))
                    )
                    nc.vector.tensor_tensor(rt4[:, c], r_ap, d_ap, OP.mult)

            # ---- R-stage reduction: halving tree + TR tail -> msg ----
            msg_t = mpool.tile([SUB, C * FEAT_DIM], f16, tag="msg")
            msg4 = msg_t[:].rearrange("p (c f) -> p c f", c=C)
            for i in range(3):
                no, m = _no(i), M_I[i]
                g = C * 8 * no
                cur = rt[i][:].rearrange("p (g m) -> p g m", g=g)
                width = m
                while width > 9:
                    half = width // 2
                    t = wpool.tile([SUB, g * half], f16, tag=f"tr{i}_{width}")
                    tv = t[:].rearrange("p (g m) -> p g m", g=g)
                    nc.vector.tensor_tensor(
                        tv, cur[:, :, 0:half], cur[:, :, half:width], OP.add)
                    cur, width = tv, half
                in_ap = cur[:, :, 0:width].rearrange(
                    "p (c g) m -> p c g m", c=C)
                out_ap = msg4[:, :, FEAT_OFF[i]:FEAT_OFF[i + 1]]
                nc.vector.tensor_reduce(
                    out_ap, in_ap, mybir.AxisListType.X, OP.add)

            # ---- combine duplicate dsts (host sel), then scatter ----
            msg2_t = mpool.tile([SUB, C * FEAT_DIM], f16, tag="msg2")
            for c in range(C):
                cmb_ps = ps_cmb.tile([SUB, FEAT_DIM], f32, tag=f"cmb{c % 2}",
                                     space="PSUM")
                nc.tensor.matmul(
                    cmb_ps[:], sel_t[:, c * SUB:(c + 1) * SUB],
                    msg_t[:, c * FEAT_DIM:(c + 1) * FEAT_DIM],
                    start=True, stop=True,
                )
                nc.scalar.copy(
                    msg2_t[:, c * FEAT_DIM:(c + 1) * FEAT_DIM], cmb_ps[:])
            for c in range(C):
                nc.gpsimd.indirect_dma_start(
                    out=out_d[:],
                    out_offset=bass.IndirectOffsetOnAxis(
                        ap=dst_t[:, c:c + 1], axis=0),
                    in_=msg2_t[:, c * FEAT_DIM:(c + 1) * FEAT_DIM],
                    in_offset=None,
                )
            if debug and s == 0:
                for name, t in (("dbg_fg", fg_t), ("dbg_r16", r16),
                                ("dbg_cy16", cy16), ("dbg_d2_0", d2[0]),
                                ("dbg_d2_1", d2[1]), ("dbg_d2_2", d2[2]),
                                ("dbg_rt_2", rt[2]), ("dbg_msg", msg_t)):
                    nc.sync.dma_start(dbg_d[name][:], t[:])
                hd = hpool.tile([H, SUPER], f16, tag="hdump")
                nc.scalar.copy(hd[:], h[:])
                nc.sync.dma_start(dbg_d["dbg_h"][:], hd[:])

    nc.finalize()
    return nc


# ----------------- host side -----------------

def _prep_consts(cc, W0, W1, W2, W3):
    W0p, W1p, W2p, W3p = fold_weights(W0, W1, W2, W3)
    cc2 = build_cc2(np.asarray(cc, dtype=np.float32)).astype(np.float16)
    centers = np.linspace(MIN_R, MAX_R, N_BASIS).astype(np.float32)
    spacing = (MAX_R - MIN_R) / (N_BASIS - 1)
    cscale = np.full((N_BASIS, 1), 1.0 / spacing, dtype=np.float32)
    cbias = (-centers / spacing).astype(np.float32).reshape(N_BASIS, 1)
    return W0p, W1p, W2p, W3p, cc2, cscale, cbias


def pack_edges(dst: np.ndarray, n_nodes: int):
    """Group edges by dst, bin-pack groups into 128-edge tiles (best-fit
    decreasing). Returns int64 [n_tiles, SUB] of edge ids, -1 for pads."""
    import bisect

    order = np.argsort(dst, kind="stable")
    ds = dst[order]
    starts = np.flatnonzero(np.r_[True, ds[1:] != ds[:-1]])
    ends = np.r_[starts[1:], len(ds)]
    runs = sorted(
        ((int(e - s), int(s), int(e)) for s, e in zip(starts, ends)),
        key=lambda r: -r[0],
    )
    assert runs[0][0] <= SUB, f"node with {runs[0][0]} > {SUB} in-edges"
    bins, rems, binidx = [], [], []
    for L, s, e in runs:
        k = bisect.bisect_left(rems, L)
        if k == len(rems):
            bins.append([(s, e)])
            r, bi = SUB - L, len(bins) - 1
        else:
            bi = binidx[k]
            r = rems[k] - L
            del rems[k], binidx[k]
            bins[bi].append((s, e))
        j = bisect.bisect_left(rems, r)
        rems.insert(j, r)
        binidx.insert(j, bi)
    tiles = []
    for b in bins:
        cur = []
        for s, e in b:
            cur.extend(order[s:e].tolist())
        cur.extend([-1] * (SUB - len(cur)))
        tiles.append(cur)
    return np.array(tiles, dtype=np.int64)


def _build_and_maps(edge_index, features, radii, rsh, cc, W0, W1, W2, W3):
    edge_index = np.asarray(edge_index)
    features = np.asarray(features, dtype=np.float32)
    radii = np.asarray(radii, dtype=np.float32)
    rsh = np.asarray(rsh, dtype=np.float32)
    n_nodes = features.shape[0]
    E = radii.shape[0]

    W0p, W1p, W2p, W3p, cc2, cscale, cbias = _prep_consts(cc, W0, W1, W2, W3)
    feat16 = features.astype(np.float16)

    src = edge_index[0].astype(np.int64)
    dst = edge_index[1].astype(np.int64)
    order = np.argsort(dst, kind="stable")

    epc = -(-E // N_CORES)
    n_super = -(-epc // SUPER)
    e_pad = n_super * SUPER

    nc = build_program(n_super, n_nodes)

    rshT = np.ascontiguousarray(rsh.T).astype(np.float16)
    in_maps = []
    for kcore in range(N_CORES):
        ids = order[kcore * epc:(kcore + 1) * epc]
        nvalid = ids.size
        idx = np.concatenate([ids, np.zeros(e_pad - nvalid, np.int64)])
        valid = np.arange(e_pad) < nvalid

        rshT_s = np.where(valid[None, :], rshT[:, idx], np.float16(0.0))
        radii_s = np.where(valid, radii[idx], np.float32(1.0)).reshape(1, -1)
        src_s = np.where(valid, src[idx], 0).astype(np.int32)
        src_r = src_s.reshape(n_super, N_SUB, SUB).transpose(0, 2, 1)
        in_maps.append(
            dict(
                rshT=np.ascontiguousarray(rshT_s),
                radii=np.ascontiguousarray(radii_s),
                srcidx=np.ascontiguousarray(src_r),
                features=feat16,
                W0p=W0p, W1p=W1p, W2p=W2p, W3p=W3p,
                CC2=cc2, cscale=cscale, cbias=cbias,
            )
        )

    post = dict(order=order, dst=dst, n_nodes=n_nodes, E=E,
                epc=epc, n_super=n_super, e_pad=e_pad)
    return nc, in_maps, post


def _segment_sum(res_list, post):
    """Host-side: unpack per-core dense messages, segment-sum by dst."""
    E, epc, e_pad = post["E"], post["epc"], post["e_pad"]
    n_super = post["n_super"]
    msgs = np.empty((E, FEAT_DIM), dtype=np.float32)
    for k, r in enumerate(res_list):
        m = np.asarray(r["out"])           # [n_super, SUB, N_SUB*FEAT]
        m = (m.reshape(n_super, SUB, N_SUB, FEAT_DIM)
             .transpose(0, 2, 1, 3).reshape(e_pad, FEAT_DIM))
        lo = k * epc
        hi = min(lo + epc, E)
        msgs[lo:hi] = m[: hi - lo]
    ds = post["dst"][post["order"]]
    starts = np.flatnonzero(np.r_[True, ds[1:] != ds[:-1]])
    sums = np.add.reduceat(msgs, starts, axis=0)
    out = np.zeros((post["n_nodes"], FEAT_DIM), dtype=np.float32)
    out[ds[starts]] = sums
    return out


def kernel(edge_index, features, radii, rsh, cc, W0, W1, W2, W3):
    from concourse.bass_utils import run_bass_kernel_spmd

    nc, in_maps, post = _build_and_maps(
        edge_index, features, radii, rsh, cc, W0, W1, W2, W3
    )
    res = run_bass_kernel_spmd(nc, in_maps, core_ids=list(range(N_CORES)))
    return _segment_sum(res.results, post)


def _install_ntff_shim():
    """Provide antenv.axon_hooks + the ctypes NTFF hook if absent."""
    import contextlib
    import ctypes
    import sys
    import types

    try:
        from antenv.axon_hooks import get_axon_ntff_profile_hook  # noqa: F401
        return
    except ImportError:
        pass

    holder = {}
    mod = types.ModuleType("antenv.axon_hooks")
    mod.set_axon_ntff_profile_hook = lambda h: holder.__setitem__("h", h)
    mod.get_axon_ntff_profile_hook = lambda: holder.get("h")
    import antenv

    sys.modules["antenv.axon_hooks"] = mod
    antenv.axon_hooks = mod

    so_path = "/opt/axon/libaxon_pjrt.so"
    try:
        lib = ctypes.CDLL(so_path)
    except OSError:
        return
    if not hasattr(lib, "axon_start_nrt_profile"):
        return
    lib.axon_start_nrt_profile.argtypes = [
        ctypes.POINTER(ctypes.c_int64),
        ctypes.c_size_t,
    ]
    lib.axon_start_nrt_profile.restype = ctypes.c_int64
    lib.axon_stop_nrt_profile.argtypes = [ctypes.c_char_p]
    lib.axon_stop_nrt_profile.restype = ctypes.c_int64

    @contextlib.contextmanager
    def _hook(output_dir, device_ids):
        import jax

        jax.devices()
        if device_ids:
            ids = (ctypes.c_int64 * len(device_ids))(*device_ids)
            rc = lib.axon_start_nrt_profile(ids, len(device_ids))
        else:
            rc = lib.axon_start_nrt_profile(None, 0)
        if rc != 0:
            raise RuntimeError(f"axon_start_nrt_profile rc={rc}")
        try:
            yield
        finally:
            n = lib.axon_stop_nrt_profile(str(output_dir).encode())
            print(f"ntff profile: {n} file(s) written to {output_dir}")

    mod.set_axon_ntff_profile_hook(_hook)


def kernel_traced(edge_index, features, radii, rsh, cc, W0, W1, W2, W3,
                  trace_cores=None, tmpdir=None):
    """Run with NTFF tracing; returns BassKernelResults."""
    _install_ntff_shim()
    from concourse import bass_utils

    bass_utils.upload_artifacts = lambda d: f"local:{d}"

    nc, in_maps, post = _build_and_maps(
        edge_index, features, radii, rsh, cc, W0, W1, W2, W3
    )
    return bass_utils.run_bass_kernel_spmd(
        nc, in_maps, core_ids=list(range(N_CORES)), trace=True,
        trace_cores=trace_cores, tmpdir=tmpdir,
    )
